# revision 58
# baseline (speedup 1.0000x reference)
"""Trainium2 Bass kernel for the ArticulatoryVQTokenizer problem (v2).

8 NeuronCores, data-parallel over tokens (16384/core). Per 512-token tile:
  mm1   : one bf16 split-3 matmul (K=44: [xh;1;xl;xh;1] x [w1h;b1h;w1h;w1l;b1l])
  LN    : reduce/Square stats, ACT Identity normalize, gpsimd *g+b
  gelu  : PE transpose (fp32) then ACT Gelu straight out of PSUM -> ghT
  mm2   : fp32, duplicated output rows ([w2|w2]) so zh/zl stack on 128 parts
  VQ    : t = 2 z.c - |c|^2 maximized via 2 bf16 matmuls:
            A: [zh;zl] x [2ch;2ch]   B: [zh;1;1] x [2cl;-cch;-ccl]
  argmax: DVE 3D reduce_max; onehot = is_equal(t, max) as fp16;
          index = ttr(onehot * iota) add-accum; histogram via col-packed
          PE ones-matmuls into one PSUM bank.
  recon : indirect-DMA gather rows of the 512-entry decoder table
          (decoder collapses: q_st == q == codebook[argmin]).
commit_loss = 0.25*mean(d2min), d2min = |z|^2 - max_t, via tiny accumulators.
Host only shards/preps inputs, sums tiny per-core partials, reshapes.
"""

import os
import numpy as np
import ml_dtypes

import concourse.bass as bass
import concourse.bacc as bacc
import concourse.tile as tile
from concourse import mybir
from concourse.bass_utils import run_bass_kernel_spmd

B, T, DIN = 16, 8192, 14
DHID, DLAT, K = 128, 64, 512
NTOK = B * T
NCORES = 8
TPC = NTOK // NCORES       # 16384
TILE = 512
NSUB = 4
NT = TPC // TILE           # 32
COMMIT_W = 0.25
LN_EPS = 1e-5
KX = 44                    # mm1 split contraction: 14+1+14+14+1

F32 = mybir.dt.float32
BF16 = mybir.dt.bfloat16
FP16 = mybir.dt.float16
I32 = mybir.dt.int32
U32 = mybir.dt.uint32

# fp32 const pack layout (columns)
C_W2D, C_G, C_B, C_ID, C_B2 = 0, 128, 640, 1152, 1280
CW_F32 = 1281
# bf16 const pack layout
BC_W1S, BC_CBA, BC_CBB, BC_ONES2 = 0, 128, 640, 1152
CW_BF = 1664

_last_exec_ns = None
_cached = None


def _build_graph():
    nc = bacc.Bacc("TRN2", target_bir_lowering=False, debug=False)

    xs_d = nc.declare_dram_parameter("xs", [KX, TPC], BF16, isOutput=False)
    cf_d = nc.declare_dram_parameter("cf32", [128, CW_F32], F32, isOutput=False)
    cb_d = nc.declare_dram_parameter("cbf16", [128, CW_BF], BF16, isOutput=False)
    io_d = nc.declare_dram_parameter("iota16", [128, K], FP16, isOutput=False)
    dect_d = nc.declare_dram_parameter("dec_table", [K, DIN], F32, isOutput=False)

    recon_d = nc.declare_dram_parameter("out_recon", [TPC, DIN], F32, isOutput=True)
    idx_d = nc.declare_dram_parameter("out_idx", [128, NT * NSUB], I32, isOutput=True)
    counts_d = nc.declare_dram_parameter("out_counts", [NSUB, K], F32, isOutput=True)
    maxv_d = nc.declare_dram_parameter("out_maxv", [128, 1], F32, isOutput=True)
    zss_d = nc.declare_dram_parameter("out_zss", [DLAT, 1], F32, isOutput=True)

    AX = mybir.AxisListType
    OP = mybir.AluOpType
    AF = mybir.ActivationFunctionType

    with tile.TileContext(nc) as tc:
        with (
            tc.tile_pool(name="cpool", bufs=1) as cpool,
            tc.tile_pool(name="wpool", bufs=2) as wpool,
            tc.tile_pool(name="spool", bufs=3) as spool,
            tc.tile_pool(name="ppool", bufs=1) as ppool,
            tc.tile_pool(name="ph", bufs=2, space="PSUM") as ph,
            tc.tile_pool(name="ptp", bufs=2, space="PSUM") as ptp,
            tc.tile_pool(name="pz", bufs=1, space="PSUM") as pz,
            tc.tile_pool(name="pt", bufs=2, space="PSUM") as pt,
            tc.tile_pool(name="pcnt", bufs=1, space="PSUM") as pcnt,
        ):
            # ---- constants (one DMA per dtype pack) ----
            cf = cpool.tile([128, CW_F32], F32)
            nc.gpsimd.dma_start(cf[:], cf_d[:, :])
            w2dup = cf[:, C_W2D : C_W2D + 128]
            g_col = cf[:, C_G : C_G + 1]
            b_col = cf[:, C_G + 1 : C_G + 2]
            ident = cf[:, C_ID : C_ID + 128]
            b2dup = cf[:, C_B2 : C_B2 + 1]
            cbf = cpool.tile([128, CW_BF], BF16)
            nc.gpsimd.dma_start(cbf[:], cb_d[:, :])
            w1s = cbf[0:KX, BC_W1S : BC_W1S + 128]
            cbA = cbf[:, BC_CBA : BC_CBA + K]
            cbB = cbf[0 : DLAT + 2, BC_CBB : BC_CBB + K]
            ones2_src = cbf[0:2, BC_ONES2 : BC_ONES2 + K]
            iota16 = cpool.tile([128, K], FP16)
            nc.gpsimd.dma_start(iota16[:], io_d[:, :])
            ones16 = cpool.tile([128, 1], FP16)
            nc.vector.memset(ones16[:], 1.0)
            ones_bf = cpool.tile([128, 1], BF16)
            nc.vector.memset(ones_bf[:], 1.0)
            mi_all = None

            # ---- persistent buffers ----
            zss_acc = ppool.tile([DLAT, 1], F32)
            nc.vector.memset(zss_acc[:], 0.0)
            idx_sb = ppool.tile([128, NT * NSUB], I32)
            idxf_all = ppool.tile([128, NT * NSUB], F32)
            mxv_all = ppool.tile([128, NT, NSUB], F32)
            mx8_all = ppool.tile([128, NT * NSUB, 8], F32)
            mi8_all = ppool.tile([128, NT * NSUB, 8], U32)
            counts_ps = pcnt.tile([128, K], F32)

            zaugB_bufs = [ppool.tile([DLAT + 2, TILE], BF16, name=f"zaugB{i}") for i in range(4)]
            for zb in zaugB_bufs:
                nc.sync.dma_start(zb[DLAT : DLAT + 2, :], ones2_src)

            # warmup: absorb const-DMA waits on PE (output clobbered by hist later)
            nc.tensor.matmul(counts_ps[:, 0:1], lhsT=ident[:], rhs=ident[:, 0:1],
                             start=True, stop=True, skip_group_check=True)
            nc.tensor.matmul(counts_ps[:, 1:2], lhsT=w1s[:], rhs=w1s[:, 0:1],
                             start=True, stop=True, skip_group_check=True)

            def front_stages(t_i):
                st = {}
                def sA():
                    tok0 = t_i * TILE
                    xt = wpool.tile([KX, TILE], BF16, tag="xt", name="xt", bufs=3)
                    nc.sync.dma_start(xt[:, :], xs_d[:, tok0 : tok0 + TILE])
                    h_ps = ph.tile([128, NSUB, DHID], F32, tag="h", name="h_ps")
                    for s in range(NSUB):
                        nc.tensor.matmul(
                            h_ps[:, s, :], lhsT=xt[:, s * 128 : (s + 1) * 128],
                            rhs=w1s[:], start=True, stop=True,
                        )
                    st["h_ps"] = h_ps
                def sB():
                    h_ps = st["h_ps"]
                    sums = spool.tile([128, NSUB], F32, tag="sums", name="sums")
                    nc.vector.tensor_reduce(sums[:], h_ps[:, :, :], axis=AX.X, op=OP.add)
                    sq_scr = wpool.tile([128, NSUB, DHID], BF16, tag="sqscr", name="sq_scr")
                    nc.scalar.activation(
                        sq_scr[:, :, :].rearrange("p a b -> p (a b)"),
                        h_ps[:, :, :].rearrange("p a b -> p (a b)"),
                        AF.Square, bias=0.0, scale=1.0,
                    )
                    ssq = spool.tile([128, NSUB], F32, tag="ssq", name="ssq")
                    nc.vector.tensor_reduce(ssq[:], sq_scr[:, :, :], axis=AX.X, op=OP.add)
                    mu = spool.tile([128, NSUB], F32, tag="mu", name="mu")
                    nc.vector.tensor_scalar(out=mu[:], in0=sums[:], scalar1=1.0 / DHID, scalar2=None, op0=OP.mult)
                    v1_ = spool.tile([128, NSUB], F32, tag="v1_", name="v1_")
                    nc.vector.tensor_tensor(out=v1_[:], in0=sums[:], in1=mu[:], op=OP.mult)
                    nc.vector.tensor_tensor(out=v1_[:], in0=ssq[:], in1=v1_[:], op=OP.subtract)
                    nc.vector.tensor_scalar(out=v1_[:], in0=v1_[:], scalar1=1.0 / DHID, scalar2=LN_EPS, op0=OP.mult, op1=OP.add)
                    rstd = spool.tile([128, NSUB], F32, tag="rstd", name="rstd")
                    ri = rstd[:].bitcast(I32)
                    nc.vector.tensor_scalar(out=ri, in0=v1_[:].bitcast(I32), scalar1=1, scalar2=None, op0=OP.arith_shift_right)
                    nc.vector.tensor_scalar(out=ri, in0=ri, scalar1=-1, scalar2=0x5F3759DF, op0=OP.mult, op1=OP.add)
                    yy = spool.tile([128, NSUB], F32, tag="yy", name="yy")
                    for _ in range(2):
                        nc.vector.tensor_tensor(out=yy[:], in0=rstd[:], in1=rstd[:], op=OP.mult)
                        nc.vector.tensor_tensor(out=yy[:], in0=v1_[:], in1=yy[:], op=OP.mult)
                        nc.vector.tensor_scalar(out=yy[:], in0=yy[:], scalar1=-0.5, scalar2=1.5, op0=OP.mult, op1=OP.add)
                        nc.vector.tensor_tensor(out=rstd[:], in0=rstd[:], in1=yy[:], op=OP.mult)
                    nmr = spool.tile([128, NSUB], F32, tag="nmr", name="nmr")
                    nc.vector.tensor_tensor(out=nmr[:], in0=mu[:], in1=rstd[:], op=OP.mult)
                    nc.vector.tensor_scalar(out=nmr[:], in0=nmr[:], scalar1=-1.0, scalar2=None, op0=OP.mult)
                    st["rstd"], st["nmr"] = rstd, nmr
                def sC():
                    h_ps, rstd, nmr = st["h_ps"], st["rstd"], st["nmr"]
                    hn = wpool.tile([128, NSUB, DHID], F32, tag="hn", name="hn")
                    for s in range(NSUB):
                        nc.scalar.activation(
                            hn[:, s, :], h_ps[:, s, :], AF.Identity,
                            bias=nmr[:, s : s + 1], scale=rstd[:, s : s + 1],
                        )
                    ghT = wpool.tile([DHID, TILE], F32, tag="ghT", name="ghT")
                    tp4 = ptp.tile([128, NSUB, 128], F32, tag="tp", name="tp4")
                    for s in range(NSUB):
                        nc.tensor.transpose(tp4[:, s, :], hn[:, s, :], ident)
                    nc.scalar.activation(
                        ghT[:, :].rearrange("p (a b) -> p a b", a=NSUB),
                        tp4[:, :, :], AF.Gelu, bias=b_col, scale=g_col,
                    )
                    st["ghT"] = ghT
                def sD():
                    ghT = st["ghT"]
                    z2_ps = pz.tile([128, TILE], F32, tag="z2", name="z2_ps")
                    for s in range(NSUB):
                        nc.tensor.matmul(
                            z2_ps[:, s * 128 : (s + 1) * 128], lhsT=w2dup,
                            rhs=ghT[:, s * 128 : (s + 1) * 128], start=True, stop=True,
                        )
                    zstack = wpool.tile([128, TILE], BF16, tag="zstack", name="zstack", bufs=4)
                    nc.scalar.activation(zstack[:, :], z2_ps[:, :], AF.Identity, bias=0.0, scale=1.0)
                    nc.vector.tensor_tensor(
                        out=zstack[DLAT:128, :], in0=z2_ps[DLAT:128, :], in1=zstack[DLAT:128, :], op=OP.subtract,
                    )
                    zaugB = zaugB_bufs[t_i % 4]
                    nc.sync.dma_start(zaugB[0:DLAT, :], zstack[0:DLAT, :])
                    zsq_scr = wpool.tile([DLAT, TILE], BF16, tag="zsqscr", name="zsq_scr")
                    zss_t = spool.tile([DLAT, 1], F32, tag="zsst", name="zss_t")
                    nc.scalar.activation(zsq_scr[:], z2_ps[0:DLAT, :], AF.Square, bias=b2dup[0:DLAT, :], scale=1.0,
                                         accum_out=zss_t[:])
                    nc.vector.tensor_tensor(out=zss_acc[:], in0=zss_acc[:], in1=zss_t[:], op=OP.add)
                    st["zstack"], st["zaugB"] = zstack, zaugB
                return [sA, sB, sC, sD], st

            def back_stages(t_i, st):
                tok0 = t_i * TILE
                rec = wpool.tile([128, NSUB, DIN], F32, tag="rec", name="rec", bufs=4)

                def mk(s):
                    def go():
                        col = t_i * NSUB + s
                        zstack, zaugB = st["zstack"], st["zaugB"]
                        t_ps = pt.tile([128, K], F32, tag="t", name="t_ps")
                        nc.tensor.matmul(t_ps[:], lhsT=zstack[:, s * 128 : (s + 1) * 128],
                                         rhs=cbA, start=True, stop=False)
                        nc.tensor.matmul(t_ps[:], lhsT=zaugB[:, s * 128 : (s + 1) * 128],
                                         rhs=cbB, start=False, stop=True)
                        mx8 = mx8_all[:, col, :]
                        nc.vector.max(out=mx8, in_=t_ps[:])
                        mi8 = mi8_all[:, col, :]
                        nc.vector.max_index(out=mi8, in_max=mx8, in_values=t_ps[:])
                        # onehot ~= Relu(1 - B*(max - t)) on ACT (B large; partial hits only
                        # for near-exact ties, negligible in the histogram)
                        obias = spool.tile([128, 1], F32, tag="obias", name="obias")
                        nc.vector.tensor_scalar(out=obias[:], in0=mx8_all[:, col, 0:1],
                                                scalar1=-8192.0, scalar2=1.0, op0=OP.mult, op1=OP.add)
                        oneh = wpool.tile([128, K], BF16, tag="oneh", name="oneh", bufs=3)
                        nc.scalar.activation(oneh[:], t_ps[:], AF.Relu, bias=obias[:], scale=8192.0)
                        nc.tensor.matmul(
                            counts_ps[0:1, :], lhsT=ones_bf[:], rhs=oneh[:],
                            start=(t_i == 0 and s == 0), stop=(t_i == NT - 1 and s == NSUB - 1),
                            skip_group_check=True,
                        )
                        nc.gpsimd.indirect_dma_start(
                            out=rec[:, s, :], out_offset=None, in_=dect_d[:, :],
                            in_offset=bass.IndirectOffsetOnAxis(ap=mi8_all[:, col, 0:1], axis=0),
                        )
                        nc.sync.dma_start(
                            recon_d[tok0 + s * 128 : tok0 + (s + 1) * 128, :], rec[:, s, :]
                        )
                    return go
                return [mk(s) for s in range(NSUB)]

            DEPTH = 3
            sts = {}
            for w in range(DEPTH):
                fs, sts[w] = front_stages(w)
                for f in fs:
                    f()
            for t_i in range(DEPTH, NT):
                fs, cur_st = front_stages(t_i)
                bs = back_stages(t_i - DEPTH, sts.pop(t_i - DEPTH))
                for i in range(NSUB):
                    fs[i]()
                    bs[i]()
                sts[t_i] = cur_st
            for w in range(NT - DEPTH, NT):
                for b in back_stages(w, sts.pop(w)):
                    b()

            # ---- finals ----
            maxv_acc = ppool.tile([128, 1], F32)
            nc.vector.tensor_reduce(maxv_acc[:], mx8_all[:, :, 0:1], axis=AX.XY, op=OP.add)
            nc.vector.tensor_copy(idx_sb[:, :], mi8_all[:, :, 0:1].rearrange("p a b -> p (a b)"))
            counts_sb = ppool.tile([128, K], F32)
            nc.scalar.copy(counts_sb[0:1, :], counts_ps[0:1, :])
            nc.sync.dma_start(counts_d[0:1, :], counts_sb[0:1, :])
            nc.sync.dma_start(maxv_d[:, :], maxv_acc[:])
            nc.sync.dma_start(zss_d[:, :], zss_acc[:])
            nc.sync.dma_start(idx_d[:, :], idx_sb[:])

    nc.finalize()
    return nc


def _np_gelu(x):
    try:
        from scipy.special import erf
        return x * 0.5 * (1.0 + erf(x / np.sqrt(2.0)))
    except Exception:
        import math
        v = np.vectorize(math.erf)
        return x * 0.5 * (1.0 + v(x / np.sqrt(2.0)))


def _bf(a):
    return np.asarray(a, ml_dtypes.bfloat16)


def _host_prep(inputs):
    w1 = np.asarray(inputs["enc_w1"], np.float32)
    b1 = np.asarray(inputs["enc_b1"], np.float32)
    g1 = np.asarray(inputs["ln1_g"], np.float32)
    be1 = np.asarray(inputs["ln1_b"], np.float32)
    w2 = np.asarray(inputs["enc_w2"], np.float32)
    b2 = np.asarray(inputs["enc_b2"], np.float32)
    cb = np.asarray(inputs["codebook"], np.float32)

    # fp32 const pack
    cf = np.zeros((128, CW_F32), np.float32)
    cf[:, C_W2D : C_W2D + 64] = w2
    cf[:, C_W2D + 64 : C_W2D + 128] = w2
    cf[:, C_G] = g1
    cf[:, C_G + 1] = be1
    cf[:, C_ID : C_ID + 128] = np.eye(128, dtype=np.float32)
    cf[0:64, C_B2] = b2
    cf[64:128, C_B2] = b2

    # bf16 const pack
    w1h = _bf(w1); w1l = _bf(w1.astype(np.float64) - w1h.astype(np.float64))
    b1h = _bf(b1); b1l = _bf(b1.astype(np.float64) - b1h.astype(np.float64))
    w1s = np.concatenate([w1h, b1h[None, :], w1h, w1l, b1l[None, :]], 0)  # [44,128]
    ch = _bf(cb); cl = _bf(cb.astype(np.float64) - ch.astype(np.float64))
    cc = (cb.astype(np.float64) ** 2).sum(-1)
    cc = cc - 2.0 * (cb.astype(np.float64) @ b2.astype(np.float64))  # fold decoder-side b2 of the encoder output
    cch = _bf(-cc); ccl = _bf(-cc - cch.astype(np.float64))
    cbA = np.concatenate([2.0 * ch.T.astype(np.float64)] * 2, 0)          # [128,512]
    cbB = np.concatenate([2.0 * cl.T.astype(np.float64), cch[None, :].astype(np.float64), ccl[None, :].astype(np.float64)], 0)  # [66,512]
    cbf = np.zeros((128, CW_BF), ml_dtypes.bfloat16)
    cbf[0:KX, BC_W1S : BC_W1S + 128] = w1s
    cbf[:, BC_CBA : BC_CBA + K] = _bf(cbA)
    cbf[0 : DLAT + 2, BC_CBB : BC_CBB + K] = _bf(cbB)
    cbf[0:2, BC_ONES2 : BC_ONES2 + K] = np.ones((2, K), ml_dtypes.bfloat16)

    iota16 = np.tile(np.arange(K, dtype=np.float16)[None, :], (128, 1))

    # decoder table
    dw1 = np.asarray(inputs["dec_w1"], np.float32)
    db1 = np.asarray(inputs["dec_b1"], np.float32)
    g2 = np.asarray(inputs["ln2_g"], np.float32)
    be2 = np.asarray(inputs["ln2_b"], np.float32)
    dw2 = np.asarray(inputs["dec_w2"], np.float32)
    db2 = np.asarray(inputs["dec_b2"], np.float32)
    h2 = cb @ dw1 + db1
    mu = h2.mean(-1, keepdims=True)
    var = h2.var(-1, keepdims=True)
    h2n = (h2 - mu) / np.sqrt(var + LN_EPS) * g2 + be2
    dec_table = (_np_gelu(h2n) @ dw2 + db2).astype(np.float32)

    return dict(cf32=cf, cbf16=cbf, iota16=iota16, dec_table=dec_table)


def _split_x(x):
    """x (NTOK,14) fp32 -> xs [44, NTOK] bf16: [xh;1;xl;xh;1]."""
    xh = _bf(x)
    xl = _bf(x.astype(np.float64) - xh.astype(np.float64))
    ones = np.ones((1, x.shape[0]), ml_dtypes.bfloat16)
    xh_t = np.ascontiguousarray(xh.T)
    xl_t = np.ascontiguousarray(xl.T)
    return np.concatenate([xh_t, ones, xl_t, xh_t, ones], 0)  # [44, NTOK]


def _ensure_ntff_hook():
    import sys, types
    try:
        from antenv import axon_hooks  # noqa
        return
    except ImportError:
        pass
    try:
        from trn_agent_boot.trn_boot import _ntff_profile_via_ctypes
        hook = _ntff_profile_via_ctypes("/opt/axon/libaxon_pjrt.so")
    except Exception:
        hook = None
    mod = types.ModuleType("antenv.axon_hooks")
    mod._hook = hook
    mod.set_axon_ntff_profile_hook = lambda h: setattr(mod, "_hook", h)
    mod.get_axon_ntff_profile_hook = lambda: mod._hook
    sys.modules["antenv.axon_hooks"] = mod


def kernel(**inputs):
    global _cached, _last_exec_ns
    x = np.asarray(inputs["x"], np.float32).reshape(NTOK, DIN)
    prep = _host_prep(inputs)
    xs = _split_x(x)

    if _cached is None:
        _cached = _build_graph()
    nc = _cached

    in_maps = []
    for c in range(NCORES):
        m = {"xs": np.ascontiguousarray(xs[:, c * TPC : (c + 1) * TPC])}
        m.update(prep)
        in_maps.append(m)

    trace = bool(int(os.environ.get("VQ_TRACE", "0")))
    if trace:
        _ensure_ntff_hook()
    res = run_bass_kernel_spmd(nc, in_maps, core_ids=list(range(NCORES)), trace=trace)
    _last_exec_ns = res.exec_time_ns

    recon = np.concatenate([res.results[c]["out_recon"] for c in range(NCORES)], 0)
    recon = recon.reshape(B, T, DIN)

    idx_parts = []
    for c in range(NCORES):
        idx_parts.append(res.results[c]["out_idx"].T.reshape(-1))
    indices = np.concatenate(idx_parts, 0).reshape(B, T).astype(np.int32)

    counts = np.zeros(K, np.float64)
    d2sum = 0.0
    for c in range(NCORES):
        counts += res.results[c]["out_counts"].astype(np.float64).sum(0)
        d2sum += float(res.results[c]["out_zss"].sum()) - float(res.results[c]["out_maxv"].sum())
    commit_loss = np.float32(COMMIT_W * d2sum / (NTOK * DLAT))
    avg = (counts / NTOK).astype(np.float32)
    perplexity = np.float32(np.exp(-np.sum(avg * np.log(avg + 1e-10))))

    return recon.astype(np.float32), indices, commit_loss, perplexity


# revision 59
# speedup vs baseline: 1.1950x; 1.1950x over previous
"""Trainium2 Bass kernel for the ArticulatoryVQTokenizer problem (v2).

8 NeuronCores, data-parallel over tokens (16384/core). Per 512-token tile:
  mm1   : one bf16 split-3 matmul (K=44: [xh;1;xl;xh;1] x [w1h;b1h;w1h;w1l;b1l])
  LN    : reduce/Square stats, ACT Identity normalize, gpsimd *g+b
  gelu  : PE transpose (fp32) then ACT Gelu straight out of PSUM -> ghT
  mm2   : fp32, duplicated output rows ([w2|w2]) so zh/zl stack on 128 parts
  VQ    : t = 2 z.c - |c|^2 maximized via 2 bf16 matmuls:
            A: [zh;zl] x [2ch;2ch]   B: [zh;1;1] x [2cl;-cch;-ccl]
  argmax: DVE 3D reduce_max; onehot = is_equal(t, max) as fp16;
          index = ttr(onehot * iota) add-accum; histogram via col-packed
          PE ones-matmuls into one PSUM bank.
  recon : indirect-DMA gather rows of the 512-entry decoder table
          (decoder collapses: q_st == q == codebook[argmin]).
commit_loss = 0.25*mean(d2min), d2min = |z|^2 - max_t, via tiny accumulators.
Host only shards/preps inputs, sums tiny per-core partials, reshapes.
"""

import os
import numpy as np
import ml_dtypes

import concourse.bass as bass
import concourse.bacc as bacc
import concourse.tile as tile
from concourse import mybir
from concourse.bass_utils import run_bass_kernel_spmd

B, T, DIN = 16, 8192, 14
DHID, DLAT, K = 128, 64, 512
NTOK = B * T
NCORES = 8
TPC = NTOK // NCORES       # 16384
TILE = 512
NSUB = 4
NT = TPC // TILE           # 32
COMMIT_W = 0.25
LN_EPS = 1e-5
KX = 44                    # mm1 split contraction: 14+1+14+14+1

F32 = mybir.dt.float32
BF16 = mybir.dt.bfloat16
FP16 = mybir.dt.float16
I32 = mybir.dt.int32
U32 = mybir.dt.uint32

# fp32 const pack layout (columns)
C_W2D, C_G, C_B, C_ID, C_B2 = 0, 128, 640, 1152, 1280
CW_F32 = 1281
# bf16 const pack layout
BC_W1S, BC_CBA, BC_CBB, BC_ONES2 = 0, 128, 640, 1152
CW_BF = 1664

_last_exec_ns = None
_cached = None


def _build_graph():
    nc = bacc.Bacc("TRN2", target_bir_lowering=False, debug=False)

    xs_d = nc.declare_dram_parameter("xs", [KX, TPC], BF16, isOutput=False)
    cf_d = nc.declare_dram_parameter("cf32", [128, CW_F32], F32, isOutput=False)
    cb_d = nc.declare_dram_parameter("cbf16", [128, CW_BF], BF16, isOutput=False)
    io_d = nc.declare_dram_parameter("iota16", [128, K], FP16, isOutput=False)
    dect_d = nc.declare_dram_parameter("dec_table", [K, DIN], F32, isOutput=False)

    recon_d = nc.declare_dram_parameter("out_recon", [TPC, DIN], F32, isOutput=True)
    idx_d = nc.declare_dram_parameter("out_idx", [128, NT * NSUB], I32, isOutput=True)
    counts_d = nc.declare_dram_parameter("out_counts", [NSUB, K], F32, isOutput=True)
    maxv_d = nc.declare_dram_parameter("out_maxv", [128, 1], F32, isOutput=True)
    zss_d = nc.declare_dram_parameter("out_zss", [DLAT, 1], F32, isOutput=True)

    AX = mybir.AxisListType
    OP = mybir.AluOpType
    AF = mybir.ActivationFunctionType

    with tile.TileContext(nc) as tc:
        with (
            tc.tile_pool(name="cpool", bufs=1) as cpool,
            tc.tile_pool(name="wpool", bufs=2) as wpool,
            tc.tile_pool(name="spool", bufs=3) as spool,
            tc.tile_pool(name="ppool", bufs=1) as ppool,
            tc.tile_pool(name="ph", bufs=2, space="PSUM") as ph,
            tc.tile_pool(name="ptp", bufs=2, space="PSUM") as ptp,
            tc.tile_pool(name="pz", bufs=1, space="PSUM") as pz,
            tc.tile_pool(name="pt", bufs=2, space="PSUM") as pt,
            tc.tile_pool(name="pcnt", bufs=1, space="PSUM") as pcnt,
        ):
            # ---- constants (one DMA per dtype pack) ----
            cf = cpool.tile([128, CW_F32], F32)
            nc.gpsimd.dma_start(cf[:], cf_d[:, :])
            w2dup = cf[:, C_W2D : C_W2D + 128]
            g_col = cf[:, C_G : C_G + 1]
            b_col = cf[:, C_G + 1 : C_G + 2]
            ident = cf[:, C_ID : C_ID + 128]
            b2dup = cf[:, C_B2 : C_B2 + 1]
            cbf = cpool.tile([128, CW_BF], BF16)
            nc.gpsimd.dma_start(cbf[:], cb_d[:, :])
            w1s = cbf[0:KX, BC_W1S : BC_W1S + 128]
            cbA = cbf[:, BC_CBA : BC_CBA + K]
            cbB = cbf[0 : DLAT + 2, BC_CBB : BC_CBB + K]
            ones2_src = cbf[0:2, BC_ONES2 : BC_ONES2 + K]
            ones_bf = cpool.tile([128, 1], BF16)
            nc.vector.memset(ones_bf[:], 1.0)

            # ---- persistent buffers ----
            zss_wide = ppool.tile([DLAT, NT], F32)
            idx_sb = ppool.tile([128, NT * NSUB], I32)
            mx8_all = ppool.tile([128, NT * NSUB, 8], F32)
            mi8_all = ppool.tile([128, NT * NSUB, 8], U32)
            counts_ps = pcnt.tile([128, K], F32)

            zaugB_bufs = [ppool.tile([DLAT + 2, TILE], BF16, name=f"zaugB{i}") for i in range(4)]
            for zb in zaugB_bufs:
                nc.sync.dma_start(zb[DLAT : DLAT + 2, :], ones2_src)

            # warmup: absorb const-DMA waits on PE (output clobbered by hist later)
            nc.tensor.matmul(counts_ps[:, 0:1], lhsT=ident[:], rhs=ident[:, 0:1],
                             start=True, stop=True, skip_group_check=True)
            nc.tensor.matmul(counts_ps[:, 1:2], lhsT=w1s[:], rhs=w1s[:, 0:1],
                             start=True, stop=True, skip_group_check=True)

            def front_stages(t_i):
                st = {}
                def sA():
                    tok0 = t_i * TILE
                    xt = wpool.tile([KX, TILE], BF16, tag="xt", name="xt", bufs=3)
                    nc.sync.dma_start(xt[:, :], xs_d[:, tok0 : tok0 + TILE])
                    h_ps = ph.tile([128, NSUB, DHID], F32, tag="h", name="h_ps")
                    for s in range(NSUB):
                        nc.tensor.matmul(
                            h_ps[:, s, :], lhsT=xt[:, s * 128 : (s + 1) * 128],
                            rhs=w1s[:], start=True, stop=True,
                        )
                    st["h_ps"] = h_ps
                def sB():
                    h_ps = st["h_ps"]
                    sums = spool.tile([128, NSUB], F32, tag="sums", name="sums")
                    nc.vector.tensor_reduce(sums[:], h_ps[:, :, :], axis=AX.X, op=OP.add)
                    sq_scr = wpool.tile([128, NSUB, DHID], BF16, tag="sqscr", name="sq_scr")
                    nc.scalar.activation(
                        sq_scr[:, :, :].rearrange("p a b -> p (a b)"),
                        h_ps[:, :, :].rearrange("p a b -> p (a b)"),
                        AF.Square, bias=0.0, scale=1.0,
                    )
                    ssq = spool.tile([128, NSUB], F32, tag="ssq", name="ssq")
                    nc.vector.tensor_reduce(ssq[:], sq_scr[:, :, :], axis=AX.X, op=OP.add)
                    mu = spool.tile([128, NSUB], F32, tag="mu", name="mu")
                    nc.vector.tensor_scalar(out=mu[:], in0=sums[:], scalar1=1.0 / DHID, scalar2=None, op0=OP.mult)
                    v1_ = spool.tile([128, NSUB], F32, tag="v1_", name="v1_")
                    nc.vector.tensor_tensor(out=v1_[:], in0=sums[:], in1=mu[:], op=OP.mult)
                    nc.vector.tensor_tensor(out=v1_[:], in0=ssq[:], in1=v1_[:], op=OP.subtract)
                    nc.vector.tensor_scalar(out=v1_[:], in0=v1_[:], scalar1=1.0 / DHID, scalar2=LN_EPS, op0=OP.mult, op1=OP.add)
                    rstd = spool.tile([128, NSUB], F32, tag="rstd", name="rstd")
                    ri = rstd[:].bitcast(I32)
                    nc.vector.tensor_scalar(out=ri, in0=v1_[:].bitcast(I32), scalar1=1, scalar2=None, op0=OP.arith_shift_right)
                    nc.vector.tensor_scalar(out=ri, in0=ri, scalar1=-1, scalar2=0x5F3759DF, op0=OP.mult, op1=OP.add)
                    yy = spool.tile([128, NSUB], F32, tag="yy", name="yy")
                    for _ in range(2):
                        nc.vector.tensor_tensor(out=yy[:], in0=rstd[:], in1=rstd[:], op=OP.mult)
                        nc.vector.tensor_tensor(out=yy[:], in0=v1_[:], in1=yy[:], op=OP.mult)
                        nc.vector.tensor_scalar(out=yy[:], in0=yy[:], scalar1=-0.5, scalar2=1.5, op0=OP.mult, op1=OP.add)
                        nc.vector.tensor_tensor(out=rstd[:], in0=rstd[:], in1=yy[:], op=OP.mult)
                    nmr = spool.tile([128, NSUB], F32, tag="nmr", name="nmr")
                    nc.vector.tensor_tensor(out=nmr[:], in0=mu[:], in1=rstd[:], op=OP.mult)
                    nc.vector.tensor_scalar(out=nmr[:], in0=nmr[:], scalar1=-1.0, scalar2=None, op0=OP.mult)
                    st["rstd"], st["nmr"] = rstd, nmr
                def sC():
                    h_ps, rstd, nmr = st["h_ps"], st["rstd"], st["nmr"]
                    hn = wpool.tile([128, NSUB, DHID], F32, tag="hn", name="hn")
                    for s in range(NSUB):
                        nc.scalar.activation(
                            hn[:, s, :], h_ps[:, s, :], AF.Identity,
                            bias=nmr[:, s : s + 1], scale=rstd[:, s : s + 1],
                        )
                    ghT = wpool.tile([DHID, TILE], F32, tag="ghT", name="ghT")
                    tp4 = ptp.tile([128, NSUB, 128], F32, tag="tp", name="tp4")
                    for s in range(NSUB):
                        nc.tensor.transpose(tp4[:, s, :], hn[:, s, :], ident)
                    nc.scalar.activation(
                        ghT[:, :].rearrange("p (a b) -> p a b", a=NSUB),
                        tp4[:, :, :], AF.Gelu, bias=b_col, scale=g_col,
                    )
                    st["ghT"] = ghT
                def sD():
                    ghT = st["ghT"]
                    z2_ps = pz.tile([128, TILE], F32, tag="z2", name="z2_ps")
                    for s in range(NSUB):
                        nc.tensor.matmul(
                            z2_ps[:, s * 128 : (s + 1) * 128], lhsT=w2dup,
                            rhs=ghT[:, s * 128 : (s + 1) * 128], start=True, stop=True,
                        )
                    zstack = wpool.tile([128, TILE], BF16, tag="zstack", name="zstack", bufs=4)
                    nc.scalar.activation(zstack[:, :], z2_ps[:, :], AF.Identity, bias=0.0, scale=1.0)
                    nc.vector.tensor_tensor(
                        out=zstack[DLAT:128, :], in0=z2_ps[DLAT:128, :], in1=zstack[DLAT:128, :], op=OP.subtract,
                    )
                    zaugB = zaugB_bufs[t_i % 4]
                    nc.sync.dma_start(zaugB[0:DLAT, :], zstack[0:DLAT, :])
                    zsq_scr = wpool.tile([DLAT, TILE], BF16, tag="zsqscr", name="zsq_scr")
                    nc.scalar.activation(zsq_scr[:], z2_ps[0:DLAT, :], AF.Square, bias=b2dup[0:DLAT, :], scale=1.0,
                                         accum_out=zss_wide[:, t_i : t_i + 1])
                    st["zstack"], st["zaugB"] = zstack, zaugB
                return [sA, sB, sC, sD], st

            def back_stages(t_i, st):
                tok0 = t_i * TILE
                rec = wpool.tile([128, NSUB, DIN], F32, tag="rec", name="rec", bufs=4)

                def mk(s):
                    def go():
                        col = t_i * NSUB + s
                        zstack, zaugB = st["zstack"], st["zaugB"]
                        t_ps = pt.tile([128, K], F32, tag="t", name="t_ps")
                        nc.tensor.matmul(t_ps[:], lhsT=zstack[:, s * 128 : (s + 1) * 128],
                                         rhs=cbA, start=True, stop=False)
                        nc.tensor.matmul(t_ps[:], lhsT=zaugB[:, s * 128 : (s + 1) * 128],
                                         rhs=cbB, start=False, stop=True)
                        mx8 = mx8_all[:, col, :]
                        nc.vector.max(out=mx8, in_=t_ps[:])
                        mi8 = mi8_all[:, col, :]
                        nc.vector.max_index(out=mi8, in_max=mx8, in_values=t_ps[:])
                        # onehot ~= Relu(1 - B*(max - t)) on ACT (B large; partial hits only
                        # for near-exact ties, negligible in the histogram)
                        obias = spool.tile([128, 1], F32, tag="obias", name="obias")
                        nc.vector.tensor_scalar(out=obias[:], in0=mx8_all[:, col, 0:1],
                                                scalar1=-8192.0, scalar2=1.0, op0=OP.mult, op1=OP.add)
                        oneh = wpool.tile([128, K], BF16, tag="oneh", name="oneh", bufs=3)
                        nc.scalar.activation(oneh[:], t_ps[:], AF.Relu, bias=obias[:], scale=8192.0)
                        nc.tensor.matmul(
                            counts_ps[0:1, :], lhsT=ones_bf[:], rhs=oneh[:],
                            start=(t_i == 0 and s == 0), stop=(t_i == NT - 1 and s == NSUB - 1),
                            skip_group_check=True,
                        )
                        nc.gpsimd.indirect_dma_start(
                            out=rec[:, s, :], out_offset=None, in_=dect_d[:, :],
                            in_offset=bass.IndirectOffsetOnAxis(ap=mi8_all[:, col, 0:1], axis=0),
                        )
                        nc.sync.dma_start(
                            recon_d[tok0 + s * 128 : tok0 + (s + 1) * 128, :], rec[:, s, :]
                        )
                    return go
                return [mk(s) for s in range(NSUB)]

            DEPTH = 3
            sts = {}
            for w in range(DEPTH):
                fs, sts[w] = front_stages(w)
                for f in fs:
                    f()
            for t_i in range(DEPTH, NT):
                fs, cur_st = front_stages(t_i)
                bs = back_stages(t_i - DEPTH, sts.pop(t_i - DEPTH))
                for i in range(NSUB):
                    fs[i]()
                    bs[i]()
                sts[t_i] = cur_st
            for w in range(NT - DEPTH, NT):
                for b in back_stages(w, sts.pop(w)):
                    b()

            # ---- finals ----
            maxv_acc = ppool.tile([128, 1], F32)
            nc.vector.tensor_reduce(maxv_acc[:], mx8_all[:, :, 0:1], axis=AX.XY, op=OP.add)
            nc.vector.tensor_copy(idx_sb[:, :], mi8_all[:, :, 0:1].rearrange("p a b -> p (a b)"))
            counts_sb = ppool.tile([128, K], F32)
            nc.scalar.copy(counts_sb[0:1, :], counts_ps[0:1, :])
            nc.sync.dma_start(counts_d[0:1, :], counts_sb[0:1, :])
            nc.sync.dma_start(maxv_d[:, :], maxv_acc[:])
            zss_acc = ppool.tile([DLAT, 1], F32)
            nc.vector.tensor_reduce(zss_acc[:], zss_wide[:, :], axis=AX.X, op=OP.add)
            nc.sync.dma_start(zss_d[:, :], zss_acc[:])
            nc.sync.dma_start(idx_d[:, :], idx_sb[:])

    nc.finalize()
    return nc


def _np_gelu(x):
    try:
        from scipy.special import erf
        return x * 0.5 * (1.0 + erf(x / np.sqrt(2.0)))
    except Exception:
        import math
        v = np.vectorize(math.erf)
        return x * 0.5 * (1.0 + v(x / np.sqrt(2.0)))


def _bf(a):
    return np.asarray(a, ml_dtypes.bfloat16)


def _host_prep(inputs):
    w1 = np.asarray(inputs["enc_w1"], np.float32)
    b1 = np.asarray(inputs["enc_b1"], np.float32)
    g1 = np.asarray(inputs["ln1_g"], np.float32)
    be1 = np.asarray(inputs["ln1_b"], np.float32)
    w2 = np.asarray(inputs["enc_w2"], np.float32)
    b2 = np.asarray(inputs["enc_b2"], np.float32)
    cb = np.asarray(inputs["codebook"], np.float32)

    # fp32 const pack
    cf = np.zeros((128, CW_F32), np.float32)
    cf[:, C_W2D : C_W2D + 64] = w2
    cf[:, C_W2D + 64 : C_W2D + 128] = w2
    cf[:, C_G] = g1
    cf[:, C_G + 1] = be1
    cf[:, C_ID : C_ID + 128] = np.eye(128, dtype=np.float32)
    cf[0:64, C_B2] = b2
    cf[64:128, C_B2] = b2

    # bf16 const pack
    w1h = _bf(w1); w1l = _bf(w1.astype(np.float64) - w1h.astype(np.float64))
    b1h = _bf(b1); b1l = _bf(b1.astype(np.float64) - b1h.astype(np.float64))
    w1s = np.concatenate([w1h, b1h[None, :], w1h, w1l, b1l[None, :]], 0)  # [44,128]
    ch = _bf(cb); cl = _bf(cb.astype(np.float64) - ch.astype(np.float64))
    cc = (cb.astype(np.float64) ** 2).sum(-1)
    cc = cc - 2.0 * (cb.astype(np.float64) @ b2.astype(np.float64))  # fold decoder-side b2 of the encoder output
    cch = _bf(-cc); ccl = _bf(-cc - cch.astype(np.float64))
    cbA = np.concatenate([2.0 * ch.T.astype(np.float64)] * 2, 0)          # [128,512]
    cbB = np.concatenate([2.0 * cl.T.astype(np.float64), cch[None, :].astype(np.float64), ccl[None, :].astype(np.float64)], 0)  # [66,512]
    cbf = np.zeros((128, CW_BF), ml_dtypes.bfloat16)
    cbf[0:KX, BC_W1S : BC_W1S + 128] = w1s
    cbf[:, BC_CBA : BC_CBA + K] = _bf(cbA)
    cbf[0 : DLAT + 2, BC_CBB : BC_CBB + K] = _bf(cbB)
    cbf[0:2, BC_ONES2 : BC_ONES2 + K] = np.ones((2, K), ml_dtypes.bfloat16)

    iota16 = np.tile(np.arange(K, dtype=np.float16)[None, :], (128, 1))

    # decoder table
    dw1 = np.asarray(inputs["dec_w1"], np.float32)
    db1 = np.asarray(inputs["dec_b1"], np.float32)
    g2 = np.asarray(inputs["ln2_g"], np.float32)
    be2 = np.asarray(inputs["ln2_b"], np.float32)
    dw2 = np.asarray(inputs["dec_w2"], np.float32)
    db2 = np.asarray(inputs["dec_b2"], np.float32)
    h2 = cb @ dw1 + db1
    mu = h2.mean(-1, keepdims=True)
    var = h2.var(-1, keepdims=True)
    h2n = (h2 - mu) / np.sqrt(var + LN_EPS) * g2 + be2
    dec_table = (_np_gelu(h2n) @ dw2 + db2).astype(np.float32)

    return dict(cf32=cf, cbf16=cbf, iota16=iota16, dec_table=dec_table)


def _split_x(x):
    """x (NTOK,14) fp32 -> xs [44, NTOK] bf16: [xh;1;xl;xh;1]."""
    xh = _bf(x)
    xl = _bf(x.astype(np.float64) - xh.astype(np.float64))
    ones = np.ones((1, x.shape[0]), ml_dtypes.bfloat16)
    xh_t = np.ascontiguousarray(xh.T)
    xl_t = np.ascontiguousarray(xl.T)
    return np.concatenate([xh_t, ones, xl_t, xh_t, ones], 0)  # [44, NTOK]


def _ensure_ntff_hook():
    import sys, types
    try:
        from antenv import axon_hooks  # noqa
        return
    except ImportError:
        pass
    try:
        from trn_agent_boot.trn_boot import _ntff_profile_via_ctypes
        hook = _ntff_profile_via_ctypes("/opt/axon/libaxon_pjrt.so")
    except Exception:
        hook = None
    mod = types.ModuleType("antenv.axon_hooks")
    mod._hook = hook
    mod.set_axon_ntff_profile_hook = lambda h: setattr(mod, "_hook", h)
    mod.get_axon_ntff_profile_hook = lambda: mod._hook
    sys.modules["antenv.axon_hooks"] = mod


def kernel(**inputs):
    global _cached, _last_exec_ns
    x = np.asarray(inputs["x"], np.float32).reshape(NTOK, DIN)
    prep = _host_prep(inputs)
    xs = _split_x(x)

    if _cached is None:
        _cached = _build_graph()
    nc = _cached

    in_maps = []
    for c in range(NCORES):
        m = {"xs": np.ascontiguousarray(xs[:, c * TPC : (c + 1) * TPC])}
        m.update(prep)
        in_maps.append(m)

    trace = bool(int(os.environ.get("VQ_TRACE", "0")))
    if trace:
        _ensure_ntff_hook()
    res = run_bass_kernel_spmd(nc, in_maps, core_ids=list(range(NCORES)), trace=trace)
    _last_exec_ns = res.exec_time_ns

    recon = np.concatenate([res.results[c]["out_recon"] for c in range(NCORES)], 0)
    recon = recon.reshape(B, T, DIN)

    idx_parts = []
    for c in range(NCORES):
        idx_parts.append(res.results[c]["out_idx"].T.reshape(-1))
    indices = np.concatenate(idx_parts, 0).reshape(B, T).astype(np.int32)

    counts = np.zeros(K, np.float64)
    d2sum = 0.0
    for c in range(NCORES):
        counts += res.results[c]["out_counts"].astype(np.float64).sum(0)
        d2sum += float(res.results[c]["out_zss"].sum()) - float(res.results[c]["out_maxv"].sum())
    commit_loss = np.float32(COMMIT_W * d2sum / (NTOK * DLAT))
    avg = (counts / NTOK).astype(np.float32)
    perplexity = np.float32(np.exp(-np.sum(avg * np.log(avg + 1e-10))))

    return recon.astype(np.float32), indices, commit_loss, perplexity
